# revision 27
# baseline (speedup 1.0000x reference)
"""CNLinkPredictor Trainium2 kernel, v2 (fused gather-transpose pipeline).

Edge-sharded across 8 NeuronCores (1024 target edges each); x, adj, and the
MLP weights are replicated. 142762ns (v1) -> 72811ns (TimelineSim + HW-verified). Per core:

  A) h = x + MLP(x) with BOTH layers in fp8 DoubleRow:
       - L1: stationary w1 [p][ksub stride 256][cout], moving xa8i
         byte-interleaved pairs (host: xa8i[p, 2n+j] = x[n, 128j+p]).
       - L2 flips orientation: stationary y1T pairs (ch halves, pair step
         512), moving w2i interleaved (host: w2i[p, 2c+j] = W2[128j+p, c]);
         bias lands via a K=1 ones-row x (b2,b2)-row matmul.
       - relu + x residual fused into ONE DVE scalar_tensor_tensor per
         psum tile: h8 = max(psL2, 0) + xr8sb, written fp8 into h8.
  B) per 128-edge group: dma_gather(transpose=True, 'mlp' gpsimd library)
     pulls 128 adjacency rows per endpoint ALREADY node-partitioned (the
     host column shuffle makes the 16-bit-granularity transpose line up
     with h8's DoubleRow block slots), one u16 bitwise-AND per group,
     then 64 DoubleRow fp8 matmuls accumulating xcnT[c, e] directly.
     No separate xbar transposes, no per-row indirect DMAs (SWDGE fixed
     cost ~1us/instr); 16 gathers stream back-to-back at full DMA bw.
  C) edge MLPs in transposed layout (bf16) per 256 edges (2 gather
     groups): xiT/xjT come from four more transposing gathers (split at
     512 idxs, see pitfalls), beta folded into xcn_w2/xcn_b2 on the
     host, z = u2 + xij never materializes (lin_w1 distributes over the
     sum in PSUM). W=256 grouping measured optimal (128 or final-split
     variants are slower: act overhead outweighs tail savings).

Scheduling notes:
  - Pool (gpsimd) queue: lib load, then all gathers upfront (g0, g1,
    xiT/xjT halves, g2..g7); pAdj bufs recycle as ANDs consume tiles.
  - SP queue: small consts, xa8i/xr8sb chunk pairs with idx16 after the
    first pair (it gates the gathers; DMA stays 100% busy ~3-67us).
  - DVE queue: 32 stage-A STTs, prod, AND(G0..7), G7 copyouts, C3 acts.
  - Act queue: L1 relus, wc load (after g0 relus), copyouts + C0-2 acts.

Hardware pitfalls (v1 set + new ones found this session):
  - dma_gather/extended insts need gpsimd library 'mlp' loaded AND
    mybir.codegen_inst_isa_subclasses(nc) before compile (raw Bass does
    not run Bacc's codegen pass; walrus fails "ISA wrong length").
  - transposing dma_gather wedges the device above 512 idxs per
    instruction (512 exact ok, 1024 NRT_EXEC_UNIT_UNRECOVERABLE).
  - gpsimd dma_start(accum_op=add) (CCE) wedges the device when
    interleaved with in-flight dma_gathers on ANY queue - do not mix;
    the STT fusion replaced it at zero cost.
  - walrus accepts at most ONE sync-wait per instruction
    (_apply_tile_patch + _split_multi_waits).
  - f32 loads before any transposing gather is in flight, f32 store
    last; everything in between is <= 2 B/elem.
  - DoubleRow stationary must be block-major (pair step % 16 == 0):
    w1 step 256, y1T step 512, h8 step 256; byte-interleaved layouts
    (xa8i, w2i, cnT) only ever appear as the MOVING operand.
  - PSUM zero regions are 2048 B; every accumulation psum tile occupies
    a full bank, so start=True zeroing never clobbers a neighbor.
  - GPSIMD cannot access PSUM - PSUM->SBUF copies stay on Act/DVE.
  - pAdj bufs=6 + moving the prod TT into the B loop (after G1) produced
    GARBAGE on HW (rel err 1e14) while TimelineSim was happy at 72659 -
    the cost model does not execute data, so scheduling tweaks that look
    free can race on silicon. Both reverted; re-verify HW after ANY
    emission-order or pool-size change before trusting it.

LANDED: nibble-packed adjacency (2 nodes/byte, markers lo=0x01 hi=0x38;
AND exact per-bit; post-AND expansion hi=(x&0x3838) one op, lo=(x&0x0101)
then in-place *56 - the BIR verifier REJECTS mixing bitwise+arith ops in
one tensor_scalar, hence the split; DR chunks iterate hi-plane first so
matmuls start before the lo MUL finishes). Halved the gather stream.
Next leads: skip the ones-matmuls when all biases are zero
(host-dispatched variant, -6.8us PE); the subnormal-psum variant (lo
plane as 0x01=2^-9 moving, combine planes at copyout with STT x512) is a
WASH - the 16 extra DVE copyout STTs cancel the saved lo-MULs; the DVE
B-stream (AND + 3 expansion ops x 8 groups, ~26us after the STT stream)
is the pacer of the 24us post-gather tail.
"""

import numpy as np
import ml_dtypes

N = 8192
C = 256
E = 8192
NCORES = 8
EL = E // NCORES          # edges per core
P = 128
NCHUNK = N // 256         # 256-node DoubleRow chunks (32)
NT = N // P               # node tiles (64)
AGRP = 512                # stage-A node group (4 tiles)
NG = N // AGRP            # stage-A groups (16)
EG = 128                  # edges per gather group
NEG = EL // EG            # gather groups per core (8)

_CACHE = {}
TRACE = False
LAST_RESULT = None


def _apply_tile_patch():
    """Split the Tile tail-drain's multi-sem wait onto individual SP nops."""
    from concourse.tile import TileContext
    from concourse.vector_clock import ScopedClock

    if getattr(TileContext, "_drain_patched", False):
        return

    def _patched(self, tick_clock, wait_clock):
        nc = self.nc
        collector = nc.sync.nop()
        wait_clock.add_sem_waits(
            collector.ins, ScopedClock({None: tick_clock.global_clock})
        )
        si = collector.ins.sync_info
        waits = list(si.on_wait) if si is not None and si.on_wait else []
        if si is not None and len(waits) > 1:
            name_to_handle = {h.name: h for h in self.sems.allocated().values()}
            si.on_wait = [waits[0]]
            for w in waits[1:]:
                op = {
                    "sem-ge-imm": "sem-ge",
                    "sem-eq-imm": "sem-eq",
                    "sem-le-imm": "sem-le",
                }.get(str(w.wait_mode), "sem-ge")
                nc.sync.nop().wait_op(name_to_handle[w.ant_name], w.wait_value, op)
        nc.sync.drain()
        nc.all_engine_barrier()
        assert self.sems is not None
        popped = nc._tile_sem_poison_stack.pop()
        assert popped is self._sem_poison
        nc.clear_and_free_semaphores(list(self.sems.allocated().values()))
        nc.all_engine_barrier()

    TileContext._drain_and_barrier = _patched
    TileContext._drain_patched = True


def _split_multi_waits(nc):
    """Hoist extra sync-waits onto same-engine NoOps (sequential waits ==
    ANDed waits); this walrus build allows one wait per instruction."""
    import concourse.mybir as mybir

    cnt = 0
    for fn in nc.m.functions:
        for bb in fn.blocks:
            out = []
            for inst in bb.instructions:
                si = getattr(inst, "sync_info", None)
                waits = list(si.on_wait) if si is not None and si.on_wait else []
                if len(waits) > 1:
                    for w in waits[:-1]:
                        nop = mybir.InstNoOp(name=f"ws-{cnt}", ins=[], outs=[])
                        cnt += 1
                        nop.engine = inst.engine
                        nop.sync_info = mybir.SyncInfo(on_wait=[w], on_update=[])
                        out.append(nop)
                    si.on_wait = [waits[-1]]
                out.append(inst)
            bb.instructions = out
    return nc


def _build(split_waits=True):
    import concourse.bass as bass
    import concourse.mybir as mybir
    from concourse.tile import TileContext

    _apply_tile_patch()

    f32 = mybir.dt.float32
    bf16 = mybir.dt.bfloat16
    fp8 = mybir.dt.float8e4
    u16 = mybir.dt.uint16
    u8 = mybir.dt.uint8
    i16 = mybir.dt.int16
    Relu = mybir.ActivationFunctionType.Relu
    Ident = mybir.ActivationFunctionType.Identity
    MUL = mybir.AluOpType.mult
    ADD = mybir.AluOpType.add
    AND = mybir.AluOpType.bitwise_and
    DR = mybir.MatmulPerfMode.DoubleRow

    nc = bass.Bass(num_swdge_queues=4, dynamic_dma_scratch_size=32768)

    # host-pretiled: xa8i[p, 2n+j] = x[n, 128j+p] (fp8, DR moving pairs)
    xa8i_d = nc.dram_tensor("xa8i", [P, 2 * N], fp8, kind="ExternalInput")
    # host-pretiled: xr8t[p, T*C + c] = x[T*128 + p, c] (fp8, h8 layout)
    xr8_d = nc.dram_tensor("xr8t", [P, 2 * N], fp8, kind="ExternalInput")
    x_d = nc.dram_tensor("x", [N, C], bf16, kind="ExternalInput")
    # nibble-packed shuffled adjacency: byte s of row r holds two nodes,
    # lo marker 0x01 for node d^-1(s) (first half), hi marker 0x38 for
    # node 4096 + d^-1(s); AND stays exact per-bit, expansion is cheap.
    adjn_d = nc.dram_tensor("adjn", [N, N // 2], u8, kind="ExternalInput")
    # wrapped i16 gather indices: idx16[16k+p, which*64 + s] =
    # tar[which, 16s+p] (16-partition wrap replicated for the 8 Q7 cores)
    idx16_d = nc.dram_tensor("idx16", [P, 2 * EL // 16], i16,
                             kind="ExternalInput")
    # fp8 stage-A weights: w1 [p,(ksub 2,cout 256)] ++ w2i [p, 2c+j]
    wa8_d = nc.dram_tensor("wa8", [P, 4 * C], fp8, kind="ExternalInput")
    onesb2_d = nc.dram_tensor("onesb2", [1, P + 2 * C], fp8,
                              kind="ExternalInput")
    # bf16 stage-C weights: [p, (which 4, k 2, cout 256)]
    wc_d = nc.dram_tensor("wc", [P, 8 * C], bf16, kind="ExternalInput")
    lin_w2_d = nc.dram_tensor("lin_w2", [C, 1], bf16, kind="ExternalInput")
    bnames = ["xlin_b1", "xcn_b1", "xcn_b2", "xij_b", "lin_b1"]
    fpk_d = nc.dram_tensor("fpk", [P, 2 * len(bnames) + 2], f32,
                           kind="ExternalInput")
    out_d = nc.dram_tensor("out", [1, EL], f32, kind="ExternalOutput")

    _gq = [0]

    def _gqn():
        q = _gq[0] % 4
        _gq[0] += 1
        return q

    from concourse import library_config

    with TileContext(nc) as tc:
        # dma_gather lives in the 'mlp' gpsimd library; load it before any
        # Pool-queue gather dispatches.
        nc.gpsimd.load_library(library_config.mlp)
        with (
            tc.tile_pool(name="const", bufs=1) as pK,
            tc.tile_pool(name="h8p", bufs=1) as pH,
            tc.tile_pool(name="adj", bufs=4) as pAdj,
            tc.tile_pool(name="cn", bufs=3) as pCn,
            tc.tile_pool(name="cnx", bufs=3) as pEx,
            tc.tile_pool(name="xcnT", bufs=1) as pXT,
            tc.tile_pool(name="xij", bufs=1) as pXi,
            tc.tile_pool(name="prod", bufs=1) as pPr,
            tc.tile_pool(name="edge", bufs=1) as pC,
        ):
            # ---- constants (SP queue; f32 fpk first, idx16 LAST so the
            # Pool gathers start only after the stage-A feeds are queued) ----
            fpk = pK.tile([P, 2 * len(bnames) + 2], f32, tag="fpk",
                          name="fpk")
            nc.sync.dma_start(out=fpk[:], in_=fpk_d[:, :])
            b_sb = {}
            for q, n in enumerate(bnames):
                b_sb[n] = fpk[:, 2 * q:2 * q + 2]
            lb2_sb = fpk[:, 11:12]

            wa8 = pK.tile([P, 4 * C], fp8, tag="wa8", name="wa8")
            nc.sync.dma_start(out=wa8[:], in_=wa8_d[:, :])
            w1_v = wa8[:, 0:2 * C].rearrange("p (j m) -> p j m", j=2)
            w2i_v = wa8[:, 2 * C:4 * C].rearrange("p (c j) -> p j c", j=2)
            onesb2 = pK.tile([1, P + 2 * C], fp8, tag="onesb2", name="onesb2")
            nc.sync.dma_start(out=onesb2[:], in_=onesb2_d[:, :])
            ones_sb = onesb2[:, 0:P]
            b2row2_sb = onesb2[:, P:P + 2 * C]

            lw2_t = pK.tile([P, 2], bf16, tag="lin_w2", name="lin_w2t")
            nc.sync.dma_start(
                out=lw2_t[:].rearrange("p (k o) -> p k o", k=2),
                in_=lin_w2_d[:, :].rearrange("(k p) o -> p k o", p=P),
            )
            lw2_sb = [lw2_t[:, 0:1], lw2_t[:, 1:2]]

            xa8i = pK.tile([P, 2 * N], fp8, tag="xa8i", name="xa8i")
            xr8sb = pK.tile([P, 2 * N], fp8, tag="xr8sb", name="xr8sb")
            idx16 = pK.tile([P, 2 * EL // 16], i16, tag="idx16",
                            name="idx16")
            for ck in range(4):
                sl = slice(ck * (N // 2), (ck + 1) * (N // 2))
                nc.sync.dma_start(out=xa8i[:, sl], in_=xa8i_d[:, sl])
                nc.sync.dma_start(out=xr8sb[:, sl], in_=xr8_d[:, sl])
                if ck == 0:
                    # gathers gate on idx16; placing it after the first
                    # chunk pair lets them interleave with the remaining
                    # stage-A feeds on the DMA engines.
                    nc.sync.dma_start(out=idx16[:], in_=idx16_d[:, :])
            xa8i_v = xa8i[:].rearrange("p (n j) -> p j n", j=2)

            # wc is loaded later on the Act queue (after g0's relus) so its
            # transfer lands behind the stage-A feeds and first gathers.
            wc_t = pK.tile([P, 8 * C], bf16, tag="wc", name="wc")
            wC_sb = {}
            for q, n in enumerate(("xcn_w1", "xcn_w2", "xij_w", "lin_w1")):
                wC_sb[n] = [wc_t[:, q * 2 * C:q * 2 * C + C],
                            wc_t[:, q * 2 * C + C:(q + 1) * 2 * C]]

            out_row = pK.tile([1, EL], f32, tag="out_row", name="out_row")

            # h8[p, T*256 + c] = h[node 128*T + p, channel c] in fp8.
            h8 = pH.tile([P, 2 * N], fp8, tag="h8", name="h8")
            h8_v = h8[:].rearrange(
                "p (ck j ch c2) -> p ck ch j c2", ck=NCHUNK, j=2, ch=2)

            # ---- transposing gathers (Pool queue) ----
            def adj_gather(which, G):
                t = pAdj.tile([P, N // 2], u8, tag=f"a{which}",
                              name=f"a{which}{G}")
                w = 0 if which == "i" else 1
                nc.gpsimd.dma_gather(
                    t[:].rearrange("p (a b) -> p a b", a=32),
                    adjn_d[:, :],
                    idx16[:, w * 64 + G * 8:w * 64 + (G + 1) * 8],
                    EG, EG, N // 2,
                    transpose=True,
                    queue_num=_gqn(),
                )
                return t

            def x_gather(which):
                # transposing gathers crash the Q7 ucode above 512 idxs
                # (observed empirically: 512 exact, 1024 wedges the device),
                # so the EL=1024 edge gather is split into two halves.
                # layout [p, (half 2, a 2, e 512)]: edge e = 512*half + e'
                t = pXi.tile([P, 2 * EL], bf16, tag=f"x{which}",
                             name=f"x{which}T")
                w = 0 if which == "i" else 1
                for h in range(2):
                    nc.gpsimd.dma_gather(
                        t[:, h * EL:(h + 1) * EL].rearrange(
                            "p (a b) -> p a b", a=2),
                        x_d[:, :],
                        idx16[:, w * 64 + h * 32:w * 64 + (h + 1) * 32],
                        EL // 2, EL // 2, C,
                        transpose=True,
                        queue_num=_gqn(),
                    )
                return t

            gath = {}
            gath[0] = (adj_gather("i", 0), adj_gather("j", 0))
            gath[1] = (adj_gather("i", 1), adj_gather("j", 1))
            xiT = x_gather("i")
            xjT = x_gather("j")
            for G in range(2, NEG):
                gath[G] = (adj_gather("i", G), adj_gather("j", G))

            # ---- stage A ----
            with tc.tile_pool(name="stA", bufs=3) as pA, \
                 tc.tile_pool(name="psA", bufs=4, space="PSUM") as psA, \
                 tc.tile_pool(name="psL2", bufs=4, space="PSUM") as psL2:
                for g in range(NG):
                    m0 = g * AGRP
                    y1T = pA.tile([P, 2 * AGRP], fp8, tag="y1T",
                                  name=f"y1T{g}")
                    for ch in range(2):
                        ps = psA.tile([P, AGRP], f32, tag="psA",
                                      name=f"psA_{g}{ch}")
                        nc.tensor.matmul(
                            ps[:], w1_v[:, :, ch * P:(ch + 1) * P],
                            xa8i_v[:, :, m0:m0 + AGRP],
                            start=True, stop=True, perf_mode=DR,
                        )
                        nc.scalar.activation(
                            y1T[:, ch * AGRP:(ch + 1) * AGRP], ps[:], Relu,
                            bias=b_sb["xlin_b1"][:, ch:ch + 1],
                        )
                    y1_v = y1T[:].rearrange("p (j n) -> p j n", j=2)
                    for half in range(2):
                        ps2 = psL2.tile([P, 2 * C], f32, tag="psL2",
                                        name=f"psL2_{g}{half}")
                        for tt in range(2):
                            t2 = 2 * half + tt
                            nc.tensor.matmul(
                                ps2[:, tt * C:(tt + 1) * C],
                                y1_v[:, :, t2 * P:(t2 + 1) * P],
                                w2i_v[:, :, :],
                                start=(tt == 0), stop=False, perf_mode=DR,
                            )
                        nc.tensor.matmul(
                            ps2[:], ones_sb[0:1, :], b2row2_sb[0:1, :],
                            start=False, stop=True,
                        )
                        c0 = (4 * g + 2 * half) * C
                        # fused relu + residual: h8 = max(ps2, 0) + x8
                        nc.vector.scalar_tensor_tensor(
                            h8[:, c0:c0 + 2 * C], ps2[:], 0.0,
                            xr8sb[:, c0:c0 + 2 * C],
                            mybir.AluOpType.max, ADD)
                    if g == 0:
                        nc.scalar.dma_start(out=wc_t[:], in_=wc_d[:, :])

            # ---- stages B + C ----
            xcnT_sb = [
                pXT.tile([P, EL], bf16, tag=f"xcnT{ch}", name=f"xcnT{ch}")
                for ch in range(2)
            ]
            prodT = pPr.tile([P, 2 * EL], bf16, tag="prodT", name="prodT")
            nc.vector.tensor_tensor(
                out=prodT[:], in0=xiT[:], in1=xjT[:], op=MUL)

            with tc.tile_pool(name="psB", bufs=4, space="PSUM") as psB, \
                 tc.tile_pool(name="psC", bufs=3, space="PSUM") as psC, \
                 tc.tile_pool(name="psO", bufs=1, space="PSUM") as psO:

                def stage_c(G):
                    W = 2 * EG
                    e0 = G * W
                    on_dve = (G == NEG // 2 - 1)

                    def act(t, ps, bname, h):
                        if on_dve:
                            nc.vector.tensor_scalar(
                                t[:], ps[:], b_sb[bname][:, h:h + 1], 0.0,
                                ADD, mybir.AluOpType.max)
                        else:
                            nc.scalar.activation(
                                t[:], ps[:], Relu,
                                bias=b_sb[bname][:, h:h + 1])

                    def mlp_layer(r0, r1, wname, bname, outtag):
                        outs = []
                        for h in range(2):
                            ps = psC.tile([P, W], f32, tag="psc",
                                          name=f"psc_{G}_{outtag}{h}")
                            nc.tensor.matmul(
                                ps[:], wC_sb[wname][0][:, h * P:(h + 1) * P],
                                r0, start=True, stop=False,
                            )
                            nc.tensor.matmul(
                                ps[:], wC_sb[wname][1][:, h * P:(h + 1) * P],
                                r1, start=False, stop=True,
                            )
                            t = pC.tile([P, W], bf16, tag=f"{outtag}{h}",
                                        name=f"{outtag}{h}_{G}")
                            act(t, ps, bname, h)
                            outs.append(t)
                        return outs

                    sl = slice(e0, e0 + W)
                    # prodT layout [p, (half 2, a 2, e 512)]
                    pr0 = (e0 // 512) * EL + (e0 % 512)
                    xijT = mlp_layer(prodT[:, pr0:pr0 + W],
                                     prodT[:, pr0 + EL // 2:pr0 + EL // 2 + W],
                                     "xij_w", "xij_b", "xijT")
                    u1T = mlp_layer(xcnT_sb[0][:, sl], xcnT_sb[1][:, sl],
                                    "xcn_w1", "xcn_b1", "u1T")
                    u2T = mlp_layer(u1T[0][:], u1T[1][:],
                                    "xcn_w2", "xcn_b2", "u2T")
                    vT = []
                    for h in range(2):
                        ps = psC.tile([P, W], f32, tag="psc",
                                      name=f"psc_{G}_vT{h}")
                        nc.tensor.matmul(
                            ps[:], wC_sb["lin_w1"][0][:, h * P:(h + 1) * P],
                            u2T[0][:], start=True, stop=False,
                        )
                        nc.tensor.matmul(
                            ps[:], wC_sb["lin_w1"][1][:, h * P:(h + 1) * P],
                            u2T[1][:], start=False, stop=False,
                        )
                        nc.tensor.matmul(
                            ps[:], wC_sb["lin_w1"][0][:, h * P:(h + 1) * P],
                            xijT[0][:], start=False, stop=False,
                        )
                        nc.tensor.matmul(
                            ps[:], wC_sb["lin_w1"][1][:, h * P:(h + 1) * P],
                            xijT[1][:], start=False, stop=True,
                        )
                        t = pC.tile([P, W], bf16, tag=f"vT{h}",
                                    name=f"vT{h}_{G}")
                        act(t, ps, "lin_b1", h)
                        vT.append(t)
                    pso = psO.tile([1, W], f32, tag="pso", name=f"pso{G}")

                    nc.tensor.matmul(
                        pso[:], lw2_sb[0][:], vT[0][:], start=True,
                        stop=False)
                    nc.tensor.matmul(
                        pso[:], lw2_sb[1][:], vT[1][:], start=False,
                        stop=True)
                    nc.scalar.activation(
                        out_row[0:1, e0:e0 + W], pso[:],
                        Ident, bias=lb2_sb[0:1, 0:1],
                    )

                for G in range(NEG):
                    ai, aj = gath[G]
                    cnp = pCn.tile([P, N // 2], u8, tag="cnp",
                                   name=f"cnp{G}")
                    nc.vector.tensor_tensor(
                        out=cnp[:].bitcast(u16),
                        in0=ai[:].bitcast(u16),
                        in1=aj[:].bitcast(u16),
                        op=AND,
                    )
                    # expand nibble planes to dense fp8 0x38 masks:
                    # lo plane (nodes < 4096, chunks 0..15): (x & 0x0101)*56
                    # hi plane (chunks 16..31): x & 0x3838 (0x38 in place)
                    cn = pEx.tile([P, N], fp8, tag="cn", name=f"cn{G}")
                    # hi plane first: the DR group iterates hi chunks first,
                    # so the matmuls start while the lo MUL is still running
                    nc.vector.tensor_scalar(
                        cn[:, N // 2:N].bitcast(u16), cnp[:].bitcast(u16),
                        0x3838, None, AND)
                    nc.vector.tensor_scalar(
                        cn[:, 0:N // 2].bitcast(u16), cnp[:].bitcast(u16),
                        0x0101, None, AND)
                    nc.vector.tensor_scalar(
                        cn[:, 0:N // 2].bitcast(u16),
                        cn[:, 0:N // 2].bitcast(u16), 56, None, MUL)
                    cn_v = cn[:].rearrange(
                        "p (ck e j) -> p ck j e", ck=NCHUNK, j=2)
                    for ch in range(2):
                        psb = psB.tile([P, EG], f32, tag="psb",
                                       name=f"psb_{G}{ch}")
                        cks = list(range(NCHUNK // 2, NCHUNK)) + \
                            list(range(NCHUNK // 2))
                        for qi, ck in enumerate(cks):
                            nc.tensor.matmul(
                                psb[:], h8_v[:, ck, ch, :, :],
                                cn_v[:, ck, :, :],
                                start=(qi == 0), stop=(qi == NCHUNK - 1),
                                perf_mode=DR,
                            )
                        if G == NEG - 1:
                            nc.vector.tensor_copy(
                                xcnT_sb[ch][:, G * EG:(G + 1) * EG], psb[:])
                        else:
                            nc.scalar.activation(
                                xcnT_sb[ch][:, G * EG:(G + 1) * EG], psb[:],
                                Ident)
                    if G % 2 == 1:
                        stage_c(G // 2)

            nc.sync.dma_start(out=out_d[:, :], in_=out_row[0:1, :])

    # Populate .instr bytes for extended-inst InstISA subclasses (the
    # PseudoReloadLibraryIndex library load + DMAGatherAnt). Raw Bass does
    # not run Bacc's codegen pass; without this walrus sees empty .instr
    # and fails codegen with "ISA wrong length".
    mybir.codegen_inst_isa_subclasses(nc)
    return _split_multi_waits(nc) if split_waits else nc


def _col_shuffle_perm():
    """d[m]: DRAM column position for original node m so the 16-bit
    transposing gather lands bytes exactly in the DoubleRow moving layout
    (node 256*ck + 128*j + p at [p, ck, byte 2e+j])."""
    m = np.arange(N)
    T = m // P
    p = m % P
    return 256 * (T // 2) + 2 * p + (T % 2)


def kernel(**inputs):
    from concourse.bass_utils import run_bass_kernel_spmd

    if "nc" not in _CACHE:
        _CACHE["nc"] = _build()
    nc = _CACHE["nc"]

    x = np.ascontiguousarray(inputs["x"], dtype=np.float32)
    ab = (np.asarray(inputs["adj"]) != 0).astype(np.uint8)
    d = _col_shuffle_perm()
    ash = np.empty_like(ab)
    ash[:, d] = ab
    adjn = np.ascontiguousarray(
        ash[:, :N // 2] * np.uint8(0x01) + ash[:, N // 2:] * np.uint8(0x38))
    tar = np.asarray(inputs["tar_ei"]).astype(np.int16)

    x8 = x.astype(ml_dtypes.float8_e4m3)
    # xa8i[p, 2n+j] = x[n, 128j+p]
    xa8i = np.ascontiguousarray(
        x8.reshape(N, 2, P).transpose(2, 0, 1).reshape(P, 2 * N))
    # xr8t[p, T*C + c] = x[T*128 + p, c]
    xr8t = np.ascontiguousarray(
        x8.reshape(NT, P, C).transpose(1, 0, 2).reshape(P, NT * C))

    def wtile(w, dt):
        # [p, (ksub 2, cout C)] from [C, C]
        return np.ascontiguousarray(
            np.asarray(w).astype(dt).reshape(2, P, C).transpose(1, 0, 2)
            .reshape(P, 2 * C))

    w2_8 = np.asarray(inputs["xlin_w2"]).astype(ml_dtypes.float8_e4m3)
    # w2i[p, 2c+j] = W2[128j+p, c]
    w2i = np.ascontiguousarray(
        w2_8.reshape(2, P, C).transpose(1, 2, 0).reshape(P, 2 * C))
    wa8 = np.concatenate(
        [wtile(inputs["xlin_w1"], ml_dtypes.float8_e4m3), w2i], axis=1)
    b2 = np.asarray(inputs["xlin_b2"], np.float32).reshape(1, C)
    onesb2 = np.concatenate(
        [np.ones((1, P), np.float32), b2, b2],
        axis=1).astype(ml_dtypes.float8_e4m3)
    beta_v = float(np.asarray(inputs["beta"]).reshape(-1)[0])
    winp = {n: np.asarray(inputs[n], np.float32) for n in
            ("xcn_w1", "xcn_w2", "xij_w", "lin_w1")}
    winp["xcn_w2"] = winp["xcn_w2"] * beta_v
    wc = np.concatenate(
        [wtile(winp[n], ml_dtypes.bfloat16)
         for n in ("xcn_w1", "xcn_w2", "xij_w", "lin_w1")], axis=1)

    def btile(b):
        return np.ascontiguousarray(
            np.asarray(b, dtype=np.float32).reshape(2, P).T)

    binp = {n: np.asarray(inputs[n], np.float32) for n in
            ("xlin_b1", "xcn_b1", "xcn_b2", "xij_b", "lin_b1")}
    binp["xcn_b2"] = binp["xcn_b2"] * beta_v
    fpk = np.concatenate(
        [btile(binp[n]) for n in
         ("xlin_b1", "xcn_b1", "xcn_b2", "xij_b", "lin_b1")] +
        [np.full((P, 1), beta_v, dtype=np.float32),
         np.full((P, 1), np.asarray(inputs["lin_b2"]).reshape(-1)[0],
                 dtype=np.float32)],
        axis=1)

    common = {
        "x": x.astype(ml_dtypes.bfloat16),
        "xa8i": xa8i,
        "xr8t": xr8t,
        "adjn": adjn,
        "wa8": wa8,
        "onesb2": onesb2,
        "wc": wc,
        "fpk": fpk,
        "lin_w2": np.ascontiguousarray(inputs["lin_w2"]).astype(
            ml_dtypes.bfloat16),
    }

    in_maps = []
    for c in range(NCORES):
        m = dict(common)
        tc_ = tar[:, c * EL:(c + 1) * EL]  # [2, EL]
        idx16 = np.empty((16, 2 * EL // 16), np.int16)
        for w in range(2):
            idx16[:, w * 64:(w + 1) * 64] = tc_[w].reshape(64, 16).T
        m["idx16"] = np.ascontiguousarray(np.tile(idx16, (8, 1)))
        in_maps.append(m)

    res = run_bass_kernel_spmd(
        nc, in_maps, core_ids=list(range(NCORES)), trace=TRACE
    )
    global LAST_RESULT
    LAST_RESULT = res
    out = np.concatenate(
        [res.results[c]["out"].reshape(EL, 1) for c in range(NCORES)], axis=0
    )
    return out.astype(np.float32)
